# revision 14
# baseline (speedup 1.0000x reference)
"""MoE experts kernel (grouped GEMM + SwiGLU) on 8 Trainium2 NeuronCores.

Problem: N=4096 tokens sorted by expert, E=8 experts, H=1024, I=2048, bf16.
  up    = gmm(hiddens, w13)            # [N, 2I]
  gated = silu(up[:, :I]) * up[:, I:]  # [N, I]
  down  = gmm(gated, w2)               # [N, H]

Sharding: expert parallelism. Core e owns expert e's weights and its
contiguous block of tokens (batch_sizes[e] each; 512 in the target
regime). No collectives; tokens are scattered/gathered on the host.

Per-core dataflow (lhsT = stationary operand of nc.tensor.matmul):
  xT    [H, T] shipped PRE-TRANSPOSED from the host (no PE transpose)
  upT   = matmul(lhsT=w13[:, chunk], rhs=xT)  -> PSUM [128, T]  (k = H)
  gatedT[c] = silu(upT_gate) * upT_up         -> SBUF bf16 chunks
  down  = matmul(lhsT=gatedT[c], rhs=w2[c])   -> PSUM [128, 512] (k = I)

Scheduling notes (walrus build: any instruction may carry at most ONE
embedded sync wait; the HWDGE queues share one 8-semaphore pool and
SWDGE has its own 8 — a 9th DMA on a pool carries a semaphore-reuse
wait, so it must be a pure load):
- ALL inputs are concatenated on the host into ONE flat partition-major
  "wall" parameter, so every load DMA moves one fat contiguous element
  per partition (DGE throughput is element-size bound: thin strided
  descriptors run several times slower).
- 5 load DMAs ride the sync HWDGE queue in exact consumption order
  (strict FIFO: a single stream gets full bandwidth and early-critical
  data is never starved by the bulk).
- Dummy PE "observer" transposes absorb DMA-progress waits so real
  matmuls only ever carry one (WAR) wait. They read garbage SBUF — no
  identity needed since their outputs are never consumed.
- A ~4.3us run of warm-up transposes (also on garbage SBUF, so they
  start right after the NEFF preamble) holds the PE busy through the
  DMA head: the HAM activity monitor needs a full busy window before
  it releases the 1.2 GHz cold-clock throttle to 2.4 GHz.
- mc 0-2 output stores ride the gpsimd SWDGE queue; the mc 3 stores
  (the tail-critical ones) take the remaining 3 HWDGE semaphores, with
  the very last PSUM group split in two so the final cast+store moves
  only 64KB.
- The tail drain chain is split one-wait-per-drain and dealt across
  the engines, latest-completing sems last.
"""

import sys

if "/opt/trn_rl_repo" not in sys.path:
    sys.path.insert(0, "/opt/trn_rl_repo")

import numpy as np
import ml_dtypes

E = 8
H = 1024
I = 2048
N = 4096
T = N // E          # tokens per expert / core
P = 128
KH = H // P         # 8  k-subtiles for mm1
NI = I // P         # 16 k-subtiles for mm2 / gated chunks
FD = 512            # matmul moving free dim (1 PSUM bank of f32)
# w13 column-slab widths per half: small first so the pipeline starts
# on minimal data while the FIFO streams the rest.
SLABS = (128, 128, 256, 512, 512, 512)
WARMUP = 72         # PE warm-up transposes through the DMA head (~7.7us)
BF16 = ml_dtypes.bfloat16

# Flat per-partition column offsets inside the "wall" parameter (bf16).
XT_OFF = 0
XT_LEN = KH * T                      # 4096
W13_OFF = XT_OFF + XT_LEN
W13_LEN = KH * 2 * I                 # 32768
W2_OFF = W13_OFF + W13_LEN
W2_LEN = NI * H                      # 16384
WALL_LEN = W2_OFF + W2_LEN           # 53248

_SLAB_BASE = []
_b = W13_OFF
for _w in SLABS:
    _SLAB_BASE.append(_b)
    _b += 2 * KH * _w

_NC_CACHE = {}


def _slab_of(c):
    """Map gated-chunk index c (0..15) -> (slab_idx, col offset in slab)."""
    for si, w in enumerate(SLABS):
        n = w // P
        if c < n:
            return si, c * P
        c -= n
    raise IndexError(c)


def _build_nc(act="silu"):
    import concourse.bass as bass
    import concourse.tile as tile
    from concourse import mybir
    from concourse.vector_clock import ScopedClock, VectorClock
    import bass_rust

    PROC_NAMES = list(bass_rust.PROC_NAMES)

    class SplitDrainTileContext(tile.TileContext):
        """Tail drain emitted as a chain of single-wait drains (the
        compiler rejects instructions with >1 embedded sync wait),
        dealt across engines with late-completing sems last."""

        def _drain_and_barrier(self, tick_clock, wait_clock):
            nc = self.nc
            gclock = tick_clock.global_clock
            n = len(gclock)

            def prio(p):
                name = PROC_NAMES[p] if p < len(PROC_NAMES) else ""
                if name.startswith("DMAHW"):
                    return (0, p)          # loads + mc3 stores
                if name.startswith("DMASW"):
                    return (2, p)          # mc0-2 stores
                if name == "Pool":
                    return (3, p)          # gates on last SW store issue
                return (1, p)              # engine ticks / sequencers

            procs = sorted((p for p in range(n) if gclock[p] > 0), key=prio)
            lanes = [nc.sync, nc.tensor, nc.vector, nc.scalar]
            for i, p in enumerate(procs):
                masked = VectorClock([gclock[q] if q == p else 0
                                      for q in range(n)])
                d = lanes[i % len(lanes)].drain()
                wait_clock.add_sem_waits(d.ins, ScopedClock({None: masked}))
            nc.all_engine_barrier()
            assert self.sems is not None
            popped = nc._tile_sem_poison_stack.pop()
            assert popped is self._sem_poison
            nc.clear_and_free_semaphores(list(self.sems.allocated().values()))

    nc = bass.Bass()
    bf = mybir.dt.bfloat16
    f32 = mybir.dt.float32

    wall = nc.declare_dram_parameter("wall", [P, WALL_LEN], bf,
                                     isOutput=False)
    # out[p, a*H + h] = down[a*P + p, h]  (host untangles)
    out = nc.declare_dram_parameter("out", [P, (T // P) * H], bf,
                                    isOutput=True)

    fn = (mybir.ActivationFunctionType.Silu if act == "silu"
          else mybir.ActivationFunctionType.Sigmoid)

    with SplitDrainTileContext(nc) as tc:
        with (
            tc.tile_pool(name="persist", bufs=1) as persist,
            tc.tile_pool(name="sgp", bufs=16) as sgp,
            tc.tile_pool(name="gtp", bufs=16) as gtp,
            tc.tile_pool(name="tch", bufs=16) as tch,
            tc.tile_pool(name="otp", bufs=1) as otp,
            tc.tile_pool(name="pst", bufs=1, space="PSUM") as pst,
            tc.tile_pool(name="ps1", bufs=2, space="PSUM") as ps1,
            tc.tile_pool(name="ps2", bufs=2, space="PSUM") as ps2,
        ):
            # ---- Load plan: single sync HWDGE queue, consumption order,
            # every DMA one fat contiguous element per partition.
            ws = persist.tile([P, WALL_LEN], bf, tag="wall")
            # slab 0 rides the second HWDGE ring (scalar) so it streams
            # concurrently with xT during the latency-critical head.
            nc.scalar.dma_start(ws[:, W13_OFF:_SLAB_BASE[1]],
                                wall[:, W13_OFF:_SLAB_BASE[1]])
            cuts = [0, W13_OFF,                       # xT
                    _SLAB_BASE[1], _SLAB_BASE[2],     # (slab0 gap) slab 1
                    _SLAB_BASE[3],                    # slab 2
                    W2_OFF,                           # slabs 3-5
                    WALL_LEN]                         # w2
            for a, b in zip(cuts, cuts[1:]):
                if a == W13_OFF:
                    continue                          # slab 0: scalar ring
                nc.sync.dma_start(ws[:, a:b], wall[:, a:b])

            def xk(k):
                return ws[:, k * T:(k + 1) * T]

            def gsl(si, k, co):
                base = _SLAB_BASE[si] + k * 2 * SLABS[si]
                return ws[:, base + co:base + co + P]

            def usl(si, k, co):
                base = _SLAB_BASE[si] + k * 2 * SLABS[si] + SLABS[si]
                return ws[:, base + co:base + co + P]

            def w2sl(kc, col, wd):
                base = W2_OFF + kc * H + col
                return ws[:, base:base + wd]

            # ---- PE warm-up + observers (all on junk SBUF) ----
            # (the framework requires a writer; one cheap gpsimd memset)
            warm = persist.tile([P, P], bf, tag="warm")
            nc.gpsimd.memset(warm[:], 0.25)
            dummy = pst.tile([P, P], bf, tag="dummy")
            for _ in range(WARMUP):
                nc.tensor.transpose(dummy[:], warm[:], warm[:])

            def observe(col):
                nc.tensor.transpose(dummy[:], ws[:, col:col + P], warm[:])

            observe(XT_OFF)      # absorbs the xT DMA (sync ring)
            observe(W13_OFF)     # absorbs the slab-0 DMA (scalar ring)

            # ---- mm1 + SwiGLU over 16 gate/up column-chunk pairs ----
            # DMA d: slab chunks it unlocks (observer before first use)
            dma_obs = {1: _SLAB_BASE[1], 2: _SLAB_BASE[2], 4: _SLAB_BASE[3]}
            slab_dma = {0: None, 1: 1, 2: 2, 3: 4, 4: 4, 5: 4}
            seen_dma = set()
            gts = []
            for c in range(NI):
                si, co = _slab_of(c)
                dmai = slab_dma[si]
                if dmai is not None and dmai not in seen_dma:
                    observe(dma_obs[dmai])
                    seen_dma.add(dmai)
                pg = ps1.tile([P, T], f32, tag="pg")
                pu = ps1.tile([P, T], f32, tag="pu")
                for k in range(KH):
                    nc.tensor.matmul(
                        pg[:], gsl(si, k, co), xk(k),
                        start=(k == 0), stop=(k == KH - 1),
                    )
                for k in range(KH):
                    nc.tensor.matmul(
                        pu[:], usl(si, k, co), xk(k),
                        start=(k == 0), stop=(k == KH - 1),
                    )
                sg = sgp.tile([P, T], bf, tag="sg")
                nc.scalar.activation(sg[:], pg[:], fn)
                # A DVE instruction may carry one sync wait: this tiny copy
                # takes the ACT wait so the gating mul below only needs PE.
                touch = tch.tile([P, 1], bf, tag="touch")
                nc.vector.tensor_copy(touch[:], sg[:, 0:1])
                gt = gtp.tile([P, T], bf, tag="gt")
                nc.vector.scalar_tensor_tensor(
                    gt[:], pu[:], 1.0, sg[:],
                    mybir.AluOpType.mult, mybir.AluOpType.mult,
                )
                gts.append(gt)

            # Observer for w2 before mm2 reads it.
            observe(W2_OFF)

            # ---- mm2: down[mc*P:, :] = gatedT.T @ w2 ----
            # obuf / out are flat [P, mc*H + h]. mc 0-2 stores go SWDGE;
            # the mc 3 stores take the 3 spare HWDGE sems, and the very
            # last PSUM group is split in two so the tail cast+store is
            # only 64KB deep.
            obuf = otp.tile([P, (T // P) * H], bf, tag="obuf")
            for mc in range(T // P):  # 4
                for nh in range(H // FD):  # 2
                    final = (mc == T // P - 1) and (nh == H // FD - 1)
                    widths = [FD] if not final else [FD // 2, FD // 2]
                    col0 = nh * FD
                    for wd in widths:
                        pdt = ps2.tile([P, wd], f32, tag="pd")
                        pdv = pdt[:]
                        for kc in range(NI):
                            nc.tensor.matmul(
                                pdv,
                                gts[kc][:, mc * P:(mc + 1) * P],
                                w2sl(kc, col0, wd),
                                start=(kc == 0), stop=(kc == NI - 1),
                            )
                        ob = obuf[:, mc * H + col0:mc * H + col0 + wd]
                        nc.vector.tensor_copy(ob, pdv)
                        eng = nc.sync if final else nc.gpsimd
                        eng.dma_start(
                            out[:, mc * H + col0:mc * H + col0 + wd], ob
                        )
                        col0 += wd

    return nc


def _get_nc():
    if "nc" not in _NC_CACHE:
        _NC_CACHE["nc"] = _build_nc()
    return _NC_CACHE["nc"]


def _prep_xt(tokens):
    """tokens [T, H] -> [P, KH*T] pre-transposed partition-major:
    xt[p, k*T + t] = tokens[t, k*P + p]."""
    xT = np.ascontiguousarray(tokens.T)          # [H, T]
    return xT.reshape(KH, P, T).transpose(1, 0, 2).reshape(P, KH * T)


def _prep_w13(w13_e):
    """w13_e [H, 2I] -> [P, KH*2I]: per-slab, per-k-subtile interleaved
    (gate cols | up cols) in kernel consumption order."""
    w4 = w13_e.reshape(KH, P, 2 * I)
    parts = []
    off = 0
    for wdt in SLABS:
        g = w4[:, :, off:off + wdt]              # [KH, P, wdt]
        u = w4[:, :, I + off:I + off + wdt]
        gu = np.concatenate([g, u], axis=2)      # [KH, P, 2*wdt]
        parts.append(gu.transpose(1, 0, 2).reshape(P, -1))
        off += wdt
    return np.concatenate(parts, axis=1)


def _prep_w2(w2_e):
    """w2_e [I, H] -> [P, NI*H] partition-major."""
    return w2_e.reshape(NI, P, H).transpose(1, 0, 2).reshape(P, NI * H)


def _make_in_map(tokens, w13_e, w2_e):
    """Per-core input dict in the kernel's current DRAM layout."""
    wallb = np.concatenate(
        [_prep_xt(np.asarray(tokens).astype(BF16)),
         _prep_w13(np.asarray(w13_e).astype(BF16)),
         _prep_w2(np.asarray(w2_e).astype(BF16))], axis=1)
    return {"wall": np.ascontiguousarray(wallb)}


def kernel(bs, hiddens, w13_weight, w2_weight, batch_sizes, **_ignored):
    from concourse.bass_utils import run_bass_kernel_spmd

    hiddens = np.asarray(hiddens)
    w13_weight = np.asarray(w13_weight)
    w2_weight = np.asarray(w2_weight)
    batch_sizes = np.asarray(batch_sizes).astype(np.int64)

    in_dtype = hiddens.dtype
    x = np.ascontiguousarray(hiddens.astype(BF16))
    w13 = np.ascontiguousarray(w13_weight.astype(BF16))
    w2 = np.ascontiguousarray(w2_weight.astype(BF16))

    assert batch_sizes.shape == (E,) and int(batch_sizes.sum()) == N, (
        "kernel compiled for 8 experts x 4096 tokens"
    )

    offsets = np.concatenate([[0], np.cumsum(batch_sizes)])
    uniform = bool((batch_sizes == T).all())

    in_maps = []
    for e in range(E):
        if uniform:
            tok = x[e * T:(e + 1) * T]
        else:
            blk = x[offsets[e]:offsets[e + 1]]
            assert blk.shape[0] <= T, "per-expert batch exceeds compiled T"
            tok = np.zeros((T, H), dtype=BF16)
            tok[: blk.shape[0]] = blk
        in_maps.append(_make_in_map(tok, w13[e], w2[e]))

    nc = _get_nc()
    results = run_bass_kernel_spmd(nc, in_maps, list(range(E))).results

    out_full = np.empty((N, H), dtype=BF16)
    for e in range(E):
        oe = np.asarray(results[e]["out"])           # [P, (T//P)*H]
        blk = oe.reshape(P, T // P, H).transpose(1, 0, 2).reshape(T, H)
        if uniform:
            out_full[e * T:(e + 1) * T] = blk
        else:
            nb = int(batch_sizes[e])
            out_full[offsets[e]:offsets[e + 1]] = blk[:nb]

    return out_full.astype(in_dtype)
